# revision 35
# baseline (speedup 1.0000x reference)
"""EnhancedSwinTransformerBlock Trainium2 kernel.

Sharding: pure data parallel over batch B=8 -> 8 NeuronCores (1 image each).
Device layout: window-grouped channel-major [C=96, T=50176] per core, bf16.
  token index t = 49*w + q  (w = window 0..1023 in raster order, q = r*7+s)

Structure: superchunks of G=8 chunks (chunk = 10 windows = 490 tokens),
processed in three phases per superchunk; all bulk ACT ops are chained in
program order so activation-table sets stay grouped (3 ACT_TABLE_LOADs per
superchunk: rsqrt set / exp set / gelu set):
  R: LN1 stats+normalize for superchunk s, LN2 for superchunk s-1
  A: window attention for s (exp softmax, tanh-based sigmoid gate),
     residual sc2 = x + gated -> SBUF ring + DRAM spill
  M: MLP for s-1 (native gelu), y kept SBUF-resident, pool accumulated
Then the tiny SE vector and a final pass out = sc2 + y*se (bf16 out).
"""

import numpy as np
import ml_dtypes
from contextlib import ExitStack

import concourse.bass as bass
from concourse import bacc
import concourse.mybir as mybir
import concourse.tile as tile
from concourse.bass_utils import run_bass_kernel_spmd

F32 = mybir.dt.float32
BF16 = mybir.dt.bfloat16
AF = mybir.ActivationFunctionType
OP = mybir.AluOpType

WS = 7
HEADS = 3
C = 96
HD = 32
HW = 224
NWS = 32                 # windows per side (224/7)
NWIN = NWS * NWS         # 1024 windows per image
WTOK = WS * WS           # 49
T = NWIN * WTOK          # 50176 tokens per core
CH_WIN = 10              # windows per chunk
CH_T = CH_WIN * WTOK     # 490
CH_PAD = 512             # padded chunk cols (psum bank size)
G = 8                    # chunks per superchunk (table-load amortization)
N_CORES = 8
EPS = 1e-5
SCALE = float(1.0 / np.sqrt(HD))
# bf16(1/96) correction so LN variance is exact
C_BF = float(np.float32(ml_dtypes.bfloat16(1.0 / C)))
VAR_CORR = float(1.0 / (C * C_BF))

_cached = {}


def _chunks():
    out = []
    w = 0
    while w < NWIN:
        nw = min(CH_WIN, NWIN - w)
        out.append((w, nw))
        w += nw
    return out


def _view4(ap2d, dims):
    """Replace the free dim of a [P, n] AP with explicit (stride,count) dims."""
    return bass.AP(ap2d.tensor, ap2d.offset, [ap2d.ap[0]] + [list(d) for d in dims])


def _build_bass():
    nc = bacc.Bacc(target_bir_lowering=False, trn_type="TRN2")

    x_d = nc.dram_tensor("xcm", [C, T], BF16, kind="ExternalInput")
    out_d = nc.dram_tensor("ocm", [C, T], BF16, kind="ExternalOutput")
    sc2_d = nc.dram_tensor("sc2_scratch", [C, T], BF16)

    wq_d = nc.dram_tensor("wq_bf", [C, C], BF16, kind="ExternalInput")
    wk_d = nc.dram_tensor("wk_bf", [C, C], BF16, kind="ExternalInput")
    wv_d = nc.dram_tensor("wv_bf", [C, C], BF16, kind="ExternalInput")
    wp_d = nc.dram_tensor("wp_bf", [C, C], BF16, kind="ExternalInput")
    wpg_d = nc.dram_tensor("wpg_bf", [C, C], BF16, kind="ExternalInput")
    w1_d = nc.dram_tensor("w1_bf", [C, 4 * C], BF16, kind="ExternalInput")
    w2_d = nc.dram_tensor("w2_bf", [3, 128, C], BF16, kind="ExternalInput")
    sw1_d = nc.dram_tensor("sew1_bf", [C, C // 4], BF16, kind="ExternalInput")
    sw2_d = nc.dram_tensor("sew2_bf", [C // 4, C], BF16, kind="ExternalInput")
    sb1_d = nc.dram_tensor("seb1", [C // 4, 1], F32, kind="ExternalInput")
    sb2_d = nc.dram_tensor("seb2", [C, 1], F32, kind="ExternalInput")

    chunks = _chunks()
    n_chunks = len(chunks)
    supers = [list(range(s, min(s + G, n_chunks))) for s in range(0, n_chunks, G)]
    n_super = len(supers)

    with tile.TileContext(nc) as tc, ExitStack() as ctx:
        singles = ctx.enter_context(tc.tile_pool(name="singles", bufs=1))

        def act(*a, **k):
            # Serialize ACT in program order so activation-table sets stay
            # grouped per phase (3 ACT_TABLE_LOADs per superchunk, not ~3
            # per chunk). Same-engine dep = pure ordering, no sem cost.
            i = nc.scalar.activation(*a, **k)
            tc.chain_iter_dep("actorder", i.ins if hasattr(i, "ins") else i)
            return i

        def load(name, shape, dtype, src):
            t = singles.tile(shape, dtype, name=name)
            nc.sync.dma_start(t[...], src)
            return t

        negones_s = singles.tile([C, C], BF16)
        nc.vector.memset(negones_s[:, :], -1.0 / C)
        onesC_s = singles.tile([C, C], BF16)
        nc.vector.memset(onesC_s[:, :], 1.0 / C)
        ones128_s = singles.tile([128, HD], BF16)
        nc.vector.memset(ones128_s[:, :], 1.0)

        wq_s = load("wq_s", [C, C], BF16, wq_d[:, :])
        wk_s = load("wk_s", [C, C], BF16, wk_d[:, :])
        wv_s = load("wv_s", [C, C], BF16, wv_d[:, :])
        wp_s = load("wp_s", [C, C], BF16, wp_d[:, :])
        wpg_s = load("wpg_s", [C, C], BF16, wpg_d[:, :])
        w1_s = load("w1_s", [C, 4 * C], BF16, w1_d[:, :])
        w2_s = load("w2_s", [128, 3, C], BF16, w2_d[:, :, :].rearrange("j p c -> p j c"))
        sw1_s = load("sw1_s", [C, C // 4], BF16, sw1_d[:, :])
        sw2_s = load("sw2_s", [C // 4, C], BF16, sw2_d[:, :])
        sb1_s = load("sb1_s", [C // 4, 1], F32, sb1_d[:, :])
        sb2_s = load("sb2_s", [C, 1], F32, sb2_d[:, :])

        eps_s = singles.tile([128, 1], F32)
        nc.vector.memset(eps_s[:, :], EPS)
        zero_s = singles.tile([128, 1], F32)
        nc.vector.memset(zero_s[:, :], 0.0)

        y_all = singles.tile([C, T], BF16)
        pool_acc = singles.tile([C, n_chunks], F32)

        p2 = ctx.enter_context(tc.tile_pool(name="p2", bufs=2))
        p2a = ctx.enter_context(tc.tile_pool(name="p2a", bufs=3))
        p3 = ctx.enter_context(tc.tile_pool(name="p3", bufs=2))
        pxh = ctx.enter_context(tc.tile_pool(name="pxh", bufs=9))
        px = ctx.enter_context(tc.tile_pool(name="px", bufs=10))
        pxh2 = ctx.enter_context(tc.tile_pool(name="pxh2", bufs=7))
        # PSUM budget (8 banks): ps1 3x [128,512] (A: q/k/v/zr/o + the
        # 2-chunk-skewed p/g, M: y; the skew keeps ring-3 reuse benign:
        # q(c+1) reuses o1(c), k/v reuse the ancient p/g of c-2)
        # + psM 1x [128,1536] (scores, MLP-h) + psR 2x [128,512] (R stats)
        ps1 = ctx.enter_context(tc.tile_pool(name="ps1", bufs=3, space="PSUM"))
        psM = ctx.enter_context(tc.tile_pool(name="psM", bufs=1, space="PSUM"))
        psR = ctx.enter_context(tc.tile_pool(name="psR", bufs=2, space="PSUM"))

        def ps_tile(name):
            return ps1.tile([128, 512], F32, tag="a", name=name)

        def psm_tile(name):
            return psM.tile([128, 1536], F32, tag="m", name=name)

        def psr_tile(name):
            return psR.tile([128, 512], F32, tag="r", name=name)

        # per-chunk SBUF tiles, keyed by chunk index
        x_tiles = {}
        sc2_tiles = {}
        xhat_tiles = {}
        xh2_tiles = {}
        oT_tiles = {}

        def load_x(group):
            for ci in group:
                w0, nw = chunks[ci]
                t0, NT = w0 * WTOK, nw * WTOK
                xt = px.tile([C, CH_T], BF16, tag="x", name="x_t")
                x_tiles[ci] = xt
                nc.sync.dma_start(xt[:, :NT], x_d[:, t0:t0 + NT])

        def phase_r(cur, prev):
            """LN1 for chunks in cur, LN2 for chunks in prev (rsqrt set)."""
            last_rs = None
            npair = max(len(cur), len(prev))
            for i in range(npair):
                ci = cur[i] if i < len(cur) else None
                pi = prev[i] if i < len(prev) else None
                rs = p2.tile([C, 2, CH_T], BF16, tag="rs", name="rs")
                if ci is not None:
                    _, nw = chunks[ci]
                    NT = nw * WTOK
                    xt = x_tiles[ci]
                    ps_nm = psr_tile("ps_nm")
                    nc.tensor.matmul(ps_nm[:C, :NT], negones_s[:, :],
                                     xt[:, :NT], start=True, stop=True)
                if pi is not None:
                    _, nwp = chunks[pi]
                    NTp = nwp * WTOK
                    st = sc2_tiles[pi]
                    ps_nm2 = psr_tile("ps_nm2")
                    nc.tensor.matmul(ps_nm2[:C, :NTp], negones_s[:, :],
                                     st[:, :NTp], start=True, stop=True)
                if ci is not None:
                    d1 = p2.tile([C, CH_T], BF16, tag="d1", name="d1")
                    nc.vector.tensor_tensor(d1[:, :NT], xt[:, :NT],
                                            ps_nm[:C, :NT], OP.add)
                    dsq1 = p2.tile([C, CH_T], BF16, tag="dsq1", name="dsq1")
                    nc.vector.tensor_tensor(dsq1[:, :NT], d1[:, :NT], d1[:, :NT],
                                            OP.mult)
                    ps_var = psr_tile("ps_var")
                    nc.tensor.matmul(ps_var[:C, :NT], onesC_s[:, :],
                                     dsq1[:, :NT], start=True, stop=True)
                if pi is not None:
                    d2 = p2.tile([C, CH_T], BF16, tag="d2", name="d2")
                    nc.vector.tensor_tensor(d2[:, :NTp], st[:, :NTp],
                                            ps_nm2[:C, :NTp], OP.add)
                    dsq2 = p2.tile([C, CH_T], BF16, tag="dsq2", name="dsq2")
                    nc.vector.tensor_tensor(dsq2[:, :NTp], d2[:, :NTp],
                                            d2[:, :NTp], OP.mult)
                    ps_var2 = psr_tile("ps_var2")
                    nc.tensor.matmul(ps_var2[:C, :NTp], onesC_s[:, :],
                                     dsq2[:, :NTp], start=True, stop=True)
                if ci is not None:
                    act(rs[:, 0, :NT], ps_var[:C, :NT],
                        AF.Abs_reciprocal_sqrt,
                        bias=eps_s[:C, :], scale=VAR_CORR)
                if pi is not None:
                    act(rs[:, 1, :NTp], ps_var2[:C, :NTp],
                        AF.Abs_reciprocal_sqrt,
                        bias=eps_s[:C, :], scale=VAR_CORR)
                if ci is not None:
                    xh = pxh.tile([C, CH_T], BF16, tag="xh", name="xh")
                    xhat_tiles[ci] = xh
                    nc.vector.tensor_tensor(xh[:, :NT], d1[:, :NT],
                                            rs[:, 0, :NT], OP.mult)
                if pi is not None:
                    xh2 = pxh2.tile([C, CH_T], BF16, tag="xh2", name="xh2")
                    xh2_tiles[pi] = xh2
                    nc.vector.tensor_tensor(xh2[:, :NTp], d2[:, :NTp],
                                            rs[:, 1, :NTp], OP.mult)
                last_rs = rs
            return last_rs

        def a_part1(ci):
            """qkv, scores, exp, Z, AV, normalize -> oT (exp set)."""
            if True:
                w0, nw = chunks[ci]
                t0, NT = w0 * WTOK, nw * WTOK
                nh2 = nw // 2
                xh = xhat_tiles.pop(ci)

                ps_q = ps_tile("ps_q")
                nc.tensor.matmul(ps_q[:C, :NT], wq_s[:, :], xh[:, :NT],
                                 start=True, stop=True)
                ps_k = ps_tile("ps_k")
                nc.tensor.matmul(ps_k[:C, :NT], wk_s[:, :], xh[:, :NT],
                                 start=True, stop=True)
                qk = p2.tile([C, 2, CH_T + 64], BF16, tag="qk", name="qk")
                nc.vector.tensor_copy(qk[:, 0, :NT], ps_q[:C, :NT])
                nc.vector.tensor_copy(qk[:, 1, :NT], ps_k[:C, :NT])
                nc.vector.memset(qk[:, 1, NT:NT + 64], 0.0)

                # v token-major, one matmul per window
                ps_v = ps_tile("ps_v")
                for s_w in range(nw):
                    p, g = s_w % 2, s_w // 2
                    nc.tensor.matmul(
                        ps_v[64 * p:64 * p + WTOK, 96 * g:96 * (g + 1)],
                        xh[:, s_w * WTOK:(s_w + 1) * WTOK],
                        wv_s[:, :], start=True, stop=True)
                v_bf = p2a.tile([128, CH_WIN // 2, C], BF16, tag="v", name="v_bf")
                nc.vector.tensor_copy(
                    v_bf[:, :nh2, :],
                    ps_v[:, :96 * nh2].rearrange("p (g c) -> p g c", c=C))

                # scores: head h in its own bank (col 512h), pair g at 49g
                ps_s = psm_tile("ps_s")
                for s_w in range(nw):
                    p, g = s_w % 2, s_w // 2
                    cs = s_w * WTOK
                    for h in range(HEADS):
                        nc.tensor.matmul(
                            ps_s[64 * p:64 * p + 64,
                                 512 * h + 49 * g:512 * h + 49 * g + WTOK],
                            qk[HD * h:HD * (h + 1), 1, cs:cs + 64],
                            qk[HD * h:HD * (h + 1), 0, cs:cs + WTOK],
                            start=True, stop=True)
                exp_bf = p2a.tile([128, 3, 245], BF16, tag="exp", name="exp_bf")
                exp_in = _view4(ps_s[:, 0:1], [(512, 3), (1, 49 * nh2)])
                exp_out = _view4(exp_bf[:, 0, 0:1], [(245, 3), (1, 49 * nh2)])
                act(exp_out, exp_in, AF.Exp, scale=SCALE,
                    bias=zero_s[:, :])

                # Z row-sums per parity in separate banks
                ps_zr = [ps_tile("ps_zr0"), ps_tile("ps_zr1")]
                for p in range(2):
                    for h in range(HEADS):
                        nc.tensor.matmul(
                            ps_zr[p][HD * h:HD * (h + 1), :49 * nh2],
                            ones128_s[64 * p:64 * p + WTOK, :],
                            exp_bf[64 * p:64 * p + WTOK, h, :49 * nh2],
                            start=True, stop=True)
                zinv = p2.tile([C, CH_T], F32, tag="zinv", name="zinv")
                for p in range(2):
                    zr_in = _view4(ps_zr[p][:C, 0:WTOK],
                                   [(WTOK, nh2), (1, WTOK)])
                    zinv_out = _view4(zinv[:, 49 * p:49 * p + WTOK],
                                      [(2 * WTOK, nh2), (1, WTOK)])
                    nc.vector.reciprocal_approx_fast(out=zinv_out, in_=zr_in)

                # AV per parity in separate banks
                ps_o = [ps_tile("ps_o0"), ps_tile("ps_o1")]
                for s_w in range(nw):
                    p, g = s_w % 2, s_w // 2
                    for h in range(HEADS):
                        nc.tensor.matmul(
                            ps_o[p][HD * h:HD * (h + 1), 49 * g:49 * g + WTOK],
                            v_bf[64 * p:64 * p + WTOK, g, HD * h:HD * (h + 1)],
                            exp_bf[64 * p:64 * p + WTOK, h,
                                   49 * g:49 * g + WTOK],
                            start=True, stop=True)
                oT_bf = p2a.tile([C, CH_T], BF16, tag="oT", name="oT_bf")
                for p in range(2):
                    o_in = _view4(ps_o[p][:C, 0:WTOK], [(WTOK, nh2), (1, WTOK)])
                    o_zinv = _view4(zinv[:, 49 * p:49 * p + WTOK],
                                    [(2 * WTOK, nh2), (1, WTOK)])
                    o_out = _view4(oT_bf[:, 49 * p:49 * p + WTOK],
                                   [(2 * WTOK, nh2), (1, WTOK)])
                    nc.vector.tensor_tensor(o_out, o_in, o_zinv, OP.mult)
                oT_tiles[ci] = oT_bf

        def a_part2(ci):
            """proj + gate (parallel matmuls on oT) + residual (exp set)."""
            if True:
                w0, nw = chunks[ci]
                t0, NT = w0 * WTOK, nw * WTOK
                oT_bf = oT_tiles.pop(ci)
                xt = x_tiles.pop(ci)
                # proj + tanh-based sigmoid gate (oP pre-halved via wp);
                # gate uses host-precomputed (w_proj @ w_gate) so both
                # matmuls read oT in parallel instead of serial p->g.
                ps_p = ps_tile("ps_p")
                nc.tensor.matmul(ps_p[:C, :NT], wp_s[:, :], oT_bf[:, :NT],
                                 start=True, stop=True)
                ps_g = ps_tile("ps_g")
                nc.tensor.matmul(ps_g[:C, :NT], wpg_s[:, :], oT_bf[:, :NT],
                                 start=True, stop=True)
                sigt = p2a.tile([C, CH_T], BF16, tag="sigt", name="sigt")
                act(sigt[:, :NT], ps_g[:C, :NT], AF.Tanh,
                    scale=0.5, bias=zero_s[:C, :])
                oP_bf = p2a.tile([C, CH_T], BF16, tag="oP", name="oP_bf")
                nc.vector.tensor_copy(oP_bf[:, :NT], ps_p[:C, :NT])
                # sc2 = x + gated = (x + oP) + oP*tanh
                g2x = p2a.tile([C, CH_T], BF16, tag="g2x", name="g2x")
                nc.gpsimd.tensor_tensor(g2x[:, :NT], sigt[:, :NT],
                                        oP_bf[:, :NT], OP.mult)
                xo = p2a.tile([C, CH_T], BF16, tag="xo", name="xo")
                nc.gpsimd.tensor_tensor(xo[:, :NT], xt[:, :NT],
                                        oP_bf[:, :NT], OP.add)
                st = px.tile([C, CH_T], BF16, tag="sc2", name="sc2_t")
                sc2_tiles[ci] = st
                nc.vector.tensor_tensor(st[:, :NT], g2x[:, :NT], xo[:, :NT],
                                        OP.add)
                nc.sync.dma_start(sc2_d[:, t0:t0 + NT], st[:, :NT])

        def phase_m(prev):
            """MLP for chunks in prev (gelu set). The y-copy is skewed
            one chunk so MLP2 doesn't insert into the ACT chain."""
            pend = None
            for ci in prev:
                w0, nw = chunks[ci]
                t0, NT = w0 * WTOK, nw * WTOK
                xh2 = xh2_tiles.pop(ci)
                ps_m = psm_tile("ps_m")
                for j in range(3):
                    nc.tensor.matmul(ps_m[:, 512 * j:512 * j + NT],
                                     w1_s[:, 128 * j:128 * (j + 1)],
                                     xh2[:, :NT], start=True, stop=True)
                h_bf = p2.tile([128, 3, CH_T], BF16, tag="h", name="h_bf")
                g_in = _view4(ps_m[:, 0:1], [(512, 3), (1, NT)])
                g_out = _view4(h_bf[:, 0, 0:1], [(CH_T, 3), (1, NT)])
                act(g_out, g_in, AF.Gelu, bias=zero_s[:, :])
                ps_y = ps_tile("ps_y")
                for j in range(3):
                    nc.tensor.matmul(ps_y[:C, :NT], w2_s[:, j, :],
                                     h_bf[:, j, :NT],
                                     start=(j == 0), stop=(j == 2))
                if pend is not None:
                    act(y_all[:, pend[1]:pend[1] + pend[2]],
                        pend[3][:C, :pend[2]], AF.Copy,
                        accum_out=pool_acc[:, pend[0]:pend[0] + 1])
                pend = (ci, t0, NT, ps_y)
            if pend is not None:
                act(y_all[:, pend[1]:pend[1] + pend[2]],
                    pend[3][:C, :pend[2]], AF.Copy,
                    accum_out=pool_acc[:, pend[0]:pend[0] + 1])

        load_x(supers[0])
        for s in range(n_super + 1):
            cur = supers[s] if s < n_super else []
            prev = supers[s - 1] if s >= 1 else []
            phase_r(cur, prev)
            if s + 1 < n_super:
                load_x(supers[s + 1])
            # part2 skewed 2 chunks behind part1: the ACT chain becomes
            # exp,exp,exp,tanh,exp,tanh,... so tanh's latency chain
            # (zr->recip->o/Z->gate-mm) has ~2 exp cadences of slack,
            # and p/g bank reuse never blocks the next chunk's zr/AV.
            SKEW = 2
            for idx, ci in enumerate(cur):
                a_part1(ci)
                if idx >= SKEW:
                    a_part2(cur[idx - SKEW])
            for ci in (cur[len(cur) - SKEW:] if SKEW > 0 else []):
                a_part2(ci)
            phase_m(prev)

        # ---- SE vector ----
        p_vec = singles.tile([C, 1], F32)
        nc.vector.reduce_sum(p_vec[:, :], pool_acc[:, :],
                             axis=mybir.AxisListType.X)
        p_bf = singles.tile([C, 1], BF16)
        act(p_bf[:, :], p_vec[:, :], AF.Copy, scale=1.0 / T)
        ps_se1 = ps_tile("ps_se1")
        nc.tensor.matmul(ps_se1[:C // 4, 0:1], sw1_s[:, :], p_bf[:, :],
                         start=True, stop=True)
        s1_bf = singles.tile([C // 4, 1], BF16)
        act(s1_bf[:, :], ps_se1[:C // 4, 0:1], AF.Silu, bias=sb1_s[:, :])
        ps_se2 = ps_tile("ps_se2")
        nc.tensor.matmul(ps_se2[:C, 0:1], sw2_s[:, :], s1_bf[:, :],
                         start=True, stop=True)
        se_vec = singles.tile([C, 1], F32)
        act(se_vec[:, :], ps_se2[:C, 0:1], AF.Sigmoid, bias=sb2_s[:, :])

        # ---- final: out = sc2 + y*se (big blocks, DVE) ----
        FB = 4 * CH_T
        t0 = 0
        while t0 < T:
            NT = min(FB, T - t0)
            sc2f = p3.tile([C, FB], BF16, tag="sc2f", name="sc2f")
            nc.sync.dma_start(sc2f[:, :NT], sc2_d[:, t0:t0 + NT])
            out_t = p3.tile([C, FB], BF16, tag="outf", name="out_t")
            nc.vector.scalar_tensor_tensor(out_t[:, :NT],
                                           y_all[:, t0:t0 + NT],
                                           se_vec[:, :], sc2f[:, :NT],
                                           OP.mult, OP.add)
            nc.sync.dma_start(out_d[:, t0:t0 + NT], out_t[:, :NT])
            t0 += NT

    nc.finalize()
    return nc


def _host_prep(inputs):
    x = np.asarray(inputs["x"], np.float32)
    ln1_g = np.asarray(inputs["ln1_g"], np.float32)
    ln1_b = np.asarray(inputs["ln1_b"], np.float32)
    w_qkv = np.asarray(inputs["w_qkv"], np.float32)
    b_qkv = np.asarray(inputs["b_qkv"], np.float32)
    w_proj = np.asarray(inputs["w_proj"], np.float32)
    w_gate = np.asarray(inputs["w_gate"], np.float32)
    ln2_g = np.asarray(inputs["ln2_g"], np.float32)
    ln2_b = np.asarray(inputs["ln2_b"], np.float32)
    mlp_w1 = np.asarray(inputs["mlp_w1"], np.float32)
    mlp_b1 = np.asarray(inputs["mlp_b1"], np.float32)
    mlp_w2 = np.asarray(inputs["mlp_w2"], np.float32)
    mlp_b2 = np.asarray(inputs["mlp_b2"], np.float32)
    se_w1 = np.asarray(inputs["se_w1"], np.float32)
    se_b1 = np.asarray(inputs["se_b1"], np.float32)
    se_w2 = np.asarray(inputs["se_w2"], np.float32)
    se_b2 = np.asarray(inputs["se_b2"], np.float32)

    bf = ml_dtypes.bfloat16

    wqkv_f = ln1_g[:, None] * w_qkv
    bqkv_f = ln1_b @ w_qkv + b_qkv
    wq, wk, wv = wqkv_f[:, :C], wqkv_f[:, C:2 * C], wqkv_f[:, 2 * C:]
    if np.abs(bqkv_f).max() > 0:
        raise NotImplementedError("nonzero qkv bias not supported")
    w1_f = ln2_g[:, None] * mlp_w1
    b1_f = ln2_b @ mlp_w1 + mlp_b1
    if np.abs(b1_f).max() > 0 or np.abs(mlp_b2).max() > 0:
        raise NotImplementedError("nonzero mlp bias not supported")

    common = {
        "wq_bf": np.ascontiguousarray(wq).astype(bf),
        "wk_bf": np.ascontiguousarray(wk).astype(bf),
        "wv_bf": np.ascontiguousarray(wv).astype(bf),
        "wp_bf": (0.5 * w_proj).astype(bf),
        "wpg_bf": np.ascontiguousarray(w_proj @ w_gate).astype(bf),
        "w1_bf": w1_f.astype(bf),
        "w2_bf": np.ascontiguousarray(mlp_w2.reshape(3, 128, C)).astype(bf),
        "sew1_bf": se_w1.astype(bf),
        "sew2_bf": se_w2.astype(bf),
        "seb1": se_b1.reshape(C // 4, 1).astype(np.float32),
        "seb2": se_b2.reshape(C, 1).astype(np.float32),
    }

    in_maps = []
    for b in range(N_CORES):
        xcm = x[b].reshape(NWS, WS, NWS, WS, C).transpose(4, 0, 2, 1, 3)
        m = dict(common)
        m["xcm"] = np.ascontiguousarray(xcm.reshape(C, T)).astype(bf)
        in_maps.append(m)
    return in_maps


def _host_post(results):
    outs = []
    for r in results:
        ocm = np.asarray(r["ocm"]).astype(np.float32).reshape(C, NWS, NWS, WS, WS)
        ob = ocm.transpose(1, 3, 2, 4, 0)
        outs.append(np.ascontiguousarray(ob.reshape(HW, HW, C)))
    return np.stack(outs, axis=0)


def kernel(**inputs) -> np.ndarray:
    import os
    if "nc" not in _cached:
        _cached["nc"] = _build_bass()
    nc = _cached["nc"]
    in_maps = _host_prep(inputs)
    trace = bool(int(os.environ.get("BASS_KERNEL_TRACE", "0")))
    res = run_bass_kernel_spmd(nc, in_maps, core_ids=list(range(N_CORES)),
                               trace=trace)
    _cached["last_results"] = res
    return _host_post(res.results).astype(np.float32)
